# revision 26
# baseline (speedup 1.0000x reference)
"""Trainium2 Bass kernel: batched QP projection via active-set direct solve.
Data parallel: 8 NeuronCores x 16 items.

fp16 single-pass linear algebra (fp32 PSUM accumulate):
  AAt = A A^T (overlapped with the A DMA; g0 = A x - b interleaved)
  M ~= inv(AAt): deg-3 Chebyshev poly init + 1 Newton-Schulz iteration,
     with iterative refinement at solve time (h = M g; h += M (g - AAt h))
     which squares the effective residual.
  z0 = x - A^T h0,  h0 = refined-solve(g0)
  2 rounds: sigma = (z<0)&mask;  S = AAt - A_sig A^T  (AAt term injected in
     PSUM via an identity-weight matmul)
     solve S w = A(sigma z0) by Chebyshev (warm started, ping-pong)
     z = z0 + A^T w   (masked columns only)
  out = x + A^T (w - h - h0) - sigma*z,  h = M (AAt w - A(sigma z))

All matvecs are column-direct on the tensor engine (per-item [128,128]
stationary blocks, symmetric matrices), no DRAM bounces. Elementwise and
PSUM-drain work is balanced across DVE / Activation / GpSimd; the big
X0 / Newton-Schulz elementwise updates are folded into PSUM via scaled
identity-weight matmuls. l1m / l1u loads are dependency-gated so the A
(at16) stream owns the DMA engines while AAt runs.
"""

import sys

for _p in ("/opt/trn_rl_repo", "/opt/pypackages"):
    if _p not in sys.path:
        sys.path.insert(0, _p)

import numpy as np
from contextlib import ExitStack

import concourse.bass as bass
import concourse.tile as tile
from concourse import mybir, bacc
from concourse.alu_op_type import AluOpType

F32 = mybir.dt.float32
F16 = mybir.dt.float16
ACT_COPY = mybir.ActivationFunctionType.Copy

B, m, n = 128, 256, 1024
NCORES = 8
I = B // NCORES      # 16
KT = n // 128        # 8
MT = m // 128        # 2
IM = I * m           # 4096

import os as _os
import json as _json

NS_ITERS = 1         # Newton-Schulz iterations (refined solves square r)
# Chebyshev iterations per active-set round (env override for experiments)
RICH = _json.loads(_os.environ.get("KRICH", "[5, 3]"))
SB_L, SB_U = 0.135, 2.01   # spectral bounds for S
NS_A, NS_B = 0.22, 2.30    # spectral bounds of AAt for NS init

_CACHE = {}


def _cheb_coeffs(l, u, iters):
    th, dl = (u + l) / 2.0, (u - l) / 2.0
    sg = th / dl
    out, rho_prev = [], None
    for k in range(iters):
        if k == 0:
            out.append((0.0, 1.0 / th))
            rho_prev = 1.0 / sg
        else:
            rho = 1.0 / (2.0 * sg - rho_prev)
            out.append((rho * rho_prev, 2.0 * rho / dl))
            rho_prev = rho
    return out  # (beta_k, gamma_k)


def _ns_init_coeffs(a, b, deg=3):
    # p(lam) = c0 + c1 lam + c2 lam^2 minimizing max |1 - lam p| on [a,b]
    import numpy.polynomial.chebyshev as C
    lam = np.linspace(a, b, 2001)
    mu = lambda x: (b + a - 2 * x) / (b - a)
    Td = C.Chebyshev.basis(deg)
    q = Td(mu(lam)) / Td(mu(0.0))
    p = np.polyfit(lam, (1 - q) / lam, deg - 1)
    return [float(v) for v in p[::-1]]


def _build(n_mk, reps=1):
    SKT = (n_mk + 127) // 128
    NM = SKT * 128
    NU = n - NM
    UKT = KT - SKT
    SI = SKT * I
    c0, c1, c2 = _ns_init_coeffs(NS_A, NS_B)
    cheb_r = [_cheb_coeffs(SB_L, SB_U, it) for it in RICH]

    nc = bacc.Bacc("TRN2", target_bir_lowering=False, debug=False, num_devices=NCORES)
    at_d = nc.declare_dram_parameter("at16", [KT, 128, IM], F16, isOutput=False)
    l1m_d = nc.declare_dram_parameter("l1m", [MT, 128, I * NM], F16, isOutput=False)
    l1u_d = nc.declare_dram_parameter("l1u", [MT, 128, I * NU], F16, isOutput=False)
    xz_d = nc.declare_dram_parameter("xz", [128, KT * I], F32, isOutput=False)
    bc_d = nc.declare_dram_parameter("bc", [128, I * MT], F32, isOutput=False)
    m01_d = nc.declare_dram_parameter("m01", [128, SI], F32, isOutput=False)
    out_d = nc.declare_dram_parameter("out", [128, KT * I], F32, isOutput=True)

    with tile.TileContext(nc) as tc, ExitStack() as ctx:
        nc = tc.nc
        ath_p = ctx.enter_context(tc.tile_pool(name="ath", bufs=1))
        l1m_p = ctx.enter_context(tc.tile_pool(name="l1m", bufs=1))
        res_p = ctx.enter_context(tc.tile_pool(name="res", bufs=1))
        l1u_p = ctx.enter_context(tc.tile_pool(name="l1u", bufs=1))
        msk_p = ctx.enter_context(tc.tile_pool(name="msk", bufs=10))
        vec_p = ctx.enter_context(tc.tile_pool(name="vec", bufs=1))
        ps_p = ctx.enter_context(tc.tile_pool(name="ps", bufs=3, space=bass.MemorySpace.PSUM))
        pv_p = ctx.enter_context(tc.tile_pool(name="pv", bufs=2, space=bass.MemorySpace.PSUM))

        AT = [ath_p.tile([128, IM], F16, name=f"at{k}", tag=f"at{k}") for k in range(KT)]
        L1M = [l1m_p.tile([128, I * NM], F16, name=f"l1m{t}", tag=f"l1m{t}") for t in range(MT)]
        AAT = [res_p.tile([128, IM], F16, name=f"aat{t}", tag=f"aat{t}") for t in range(MT)]
        MH = [res_p.tile([128, IM], F16, name=f"mh{t}", tag=f"mh{t}") for t in range(MT)]
        SH = [res_p.tile([128, IM], F16, name=f"sh{t}", tag=f"sh{t}") for t in range(MT)]
        HB = [res_p.tile([128, IM], F16, name=f"hb{t}", tag=f"hb{t}") for t in range(MT)]
        LU = [l1u_p.tile([128, 8 * NU], F16, name=f"lu{j}", tag=f"lu{j}") for j in range(4)]

        def vt(name, cols, dt=F32):
            return vec_p.tile([128, cols], dt, name=name, tag=name)

        xzv = vt("xzv", KT * I)
        xz16 = vt("xz16", KT * I, F16)
        bcol = vt("bcol", I * MT)
        m01v = vt("m01v", SI)
        g0v = vt("g0v", I * MT)
        h0v = vt("h0v", I * MT)
        u16 = vt("u16", I * MT, F16)
        r16 = vt("r16", I * MT, F16)
        wa = vt("wa", I * MT)
        wb = vt("wb", I * MT)  # holds the Chebyshev difference d_k
        wtmp = vt("wtmp", I * MT)
        rtmp = vt("rtmp", I * MT)
        rhsc = vt("rhsc", I * MT)
        wfv = vt("wfv", I * MT)
        w16 = vt("w16", I * MT, F16)
        g16 = vt("g16", I * MT, F16)
        h016 = vt("h016", I * MT, F16)
        wf16 = vt("wf16", I * MT, F16)
        z0m = vt("z0m", SI)
        zm = vt("zm", SI)
        sig = vt("sig", SI)
        msig = vt("msig", SI)
        tmpn = vt("tmpn", SI)
        su16 = vt("su16", SI, F16)
        nsz16 = vt("nsz16", SI, F16)
        outv = vt("outv", KT * I)
        id128 = vt("id128", 128, F16)    # [128,128] fp16 identity
        id2 = vt("id2", 128, F16)        # 2.0-scaled identity (NS fold)
        idc1 = vt("idc1", 128, F16)      # (c1/c2)-scaled identity (X0 fold)
        # (c0/c2)-scaled identity blocks for the X0 PE-injection
        IDX = [vt(f"idx{t}", m, F16) for t in range(MT)]

        def emit_body():
            # ---------- loads: small vectors + AT first; l1m/l1u gated ----------
            for kt in range(KT):
                nc.sync.dma_start(out=AT[kt][:, 0:2048], in_=at_d[kt][:, 0:2048])
            nc.sync.dma_start(out=xzv[:], in_=xz_d[:])
            nc.sync.dma_start(out=bcol[:], in_=bc_d[:])
            nc.sync.dma_start(out=m01v[:], in_=m01_d[:])
            for kt in range(KT):
                nc.sync.dma_start(out=AT[kt][:, 2048:4096],
                                  in_=at_d[kt][:, 2048:4096])
            nc.gpsimd.memset(id128[:], 1.0)
            nc.gpsimd.affine_select(id128[:], id128[:], [[1, 128]], AluOpType.is_equal,
                                    0.0, base=0, channel_multiplier=-1)
            nc.gpsimd.memset(id2[:], 2.0)
            nc.gpsimd.affine_select(id2[:], id2[:], [[1, 128]], AluOpType.is_equal,
                                    0.0, base=0, channel_multiplier=-1)
            nc.gpsimd.memset(idc1[:], c1 / c2)
            nc.gpsimd.affine_select(idc1[:], idc1[:], [[1, 128]], AluOpType.is_equal,
                                    0.0, base=0, channel_multiplier=-1)
            for t in range(MT):
                nc.gpsimd.memset(IDX[t][:], c0 / c2)
                nc.gpsimd.affine_select(IDX[t][:], IDX[t][:], [[1, m]],
                                        AluOpType.is_equal, 0.0,
                                        base=-t * 128, channel_multiplier=-1)
            nc.vector.tensor_copy(xz16[:], xzv[:])
            nc.gpsimd.memset(wa[:], 0.0)
            nc.gpsimd.memset(wb[:], 0.0)

            # PSUM-drain copies alternate DVE / Activation
            _alt = [0]

            def drain(dst, src, scale=None):
                _alt[0] ^= 1
                if _alt[0]:
                    if scale is None:
                        nc.vector.tensor_copy(dst, src)
                    else:
                        nc.vector.tensor_scalar(dst, src, scale, None, AluOpType.mult)
                else:
                    if scale is None:
                        nc.scalar.copy(dst, src)
                    else:
                        nc.scalar.activation(dst, src, ACT_COPY, scale=scale)

            # ---------- batched [m x m] = lhs^T-blocks @ rhs products ----------
            def mm_groups(lhs, rhs_, kts, post, groups, inject=None):
                nk = len(kts)
                for t, g0 in groups:
                    ps = ps_p.tile([128, 1024], F32, name="psb", tag="psb")
                    if inject is not None:
                        for gi in range(4):
                            inject(t, g0 + gi, gi, ps)
                    for ki, kt in enumerate(kts):
                        for gi in range(4):
                            i = g0 + gi
                            nc.tensor.matmul(
                                ps[:, gi * m:(gi + 1) * m],
                                lhs[kt][:, i * m + t * 128: i * m + t * 128 + 128],
                                rhs_[kt][:, i * m:(i + 1) * m],
                                start=(inject is None and ki == 0 and gi % 2 == 0),
                                stop=(ki == nk - 1 and gi % 2 == 1))
                    post(t, g0, ps)

            GRPS_A = [(0, 0), (1, 0), (0, 4), (1, 4)]
            GRPS_B = [(0, 8), (1, 8), (0, 12), (1, 12)]

            def pv():
                return pv_p.tile([128, 512], F32, name="pvb", tag="pvb")

            # ---------- column-direct matvec helpers ----------
            def mv_sym(ps, W, v16, items=None, base=0):
                # ps[:, (i-base)*MT+cb] = (W_i v_i)[cb-block]; W symmetric
                for i in (items if items is not None else range(I)):
                    for cb in range(MT):
                        col = (i - base) * MT + cb
                        for rb in range(MT):
                            nc.tensor.matmul(
                                ps[:, col:col + 1],
                                W[rb][:, i * m + cb * 128: i * m + cb * 128 + 128],
                                v16[:, i * MT + rb: i * MT + rb + 1],
                                start=(rb == 0), stop=(rb == MT - 1))

            def mv_dn(ps, v16n, kts, items=None):
                # ps[:, i*MT+cb] = (A_i v_i)[cb-block], contraction over n-blocks kts
                nk = len(kts)
                for i in (items if items is not None else range(I)):
                    for cb in range(MT):
                        col = i * MT + cb
                        for ki, kt in enumerate(kts):
                            nc.tensor.matmul(
                                ps[:, col:col + 1],
                                AT[kt][:, i * m + cb * 128: i * m + cb * 128 + 128],
                                v16n[:, kt * I + i: kt * I + i + 1],
                                start=(ki == 0), stop=(ki == nk - 1))

            def mv_up(ps, w16_, kts, items=None):
                # ps[:, kt*I+i] = (A_i^T w_i)[kt-block] over masked columns
                for i in (items if items is not None else range(I)):
                    for kt in kts:
                        col = kt * I + i
                        for rb in range(MT):
                            nc.tensor.matmul(
                                ps[:, col:col + 1],
                                L1M[rb][:, i * NM + kt * 128: i * NM + kt * 128 + 128],
                                w16_[:, i * MT + rb: i * MT + rb + 1],
                                start=(rb == 0), stop=(rb == MT - 1))

            # ---------- AAt + g0 interleaved with the AT DMA ----------
            def post_aat(t, g0, ps):
                drain(AAT[t][:, g0 * m:(g0 + 4) * m], ps[:])

            mm_groups(AT, AT, list(range(KT)), post_aat, GRPS_A)
            # gate l1m load on first AAt output (keeps DMA free for AT)
            for t in range(MT):
                nc.scalar.copy(L1M[t][0:1, 0:1], AAT[0][0:1, 0:1])
                nc.sync.dma_start(out=L1M[t][:], in_=l1m_d[t])
            # g0 = A x - b for items 0-7 (AT first halves only)
            psgA = pv()
            mv_dn(psgA, xz16, list(range(KT)), items=range(8))
            mm_groups(AT, AT, list(range(KT)), post_aat, GRPS_B)
            psgB = pv()
            mv_dn(psgB, xz16, list(range(KT)), items=range(8, I))
            nc.vector.tensor_tensor(g0v[:, 0:16], psgA[:, 0:16], bcol[:, 0:16],
                                    AluOpType.subtract)
            nc.vector.tensor_tensor(g0v[:, 16:32], psgB[:, 16:32], bcol[:, 16:32],
                                    AluOpType.subtract)
            nc.scalar.copy(g16[:], g0v[:])

            # ---------- X0 = c0 I + c1 AAt + c2 AAt^2 ----------
            # identity and c1-AAt terms injected in PSUM via identity-weight
            # matmuls; drain is a single scaled copy. Result into X0dst.
            Xbufs = [MH, SH]
            X0dst = Xbufs[NS_ITERS % 2]

            def inj_x0(t, i, gi, ps):
                sl = slice(gi * m, (gi + 1) * m)
                nc.tensor.matmul(ps[:, sl], id128[:], IDX[t][:],
                                 start=(gi % 2 == 0), stop=False)
                nc.tensor.matmul(ps[:, sl], idc1[:],
                                 AAT[t][:, i * m:(i + 1) * m],
                                 start=False, stop=False)

            def post_x0(t, g0, ps):
                drain(X0dst[t][:, g0 * m:(g0 + 4) * m], ps[:], scale=c2)

            mm_groups(AAT, AAT, [0, 1], post_x0, GRPS_A + GRPS_B, inject=inj_x0)

            # ---------- Newton-Schulz: X <- X (2I - AAt X) ----------
            # H-pass stores -(AAt X); X-pass injects 2X in PSUM, drains plain.
            for it in range(NS_ITERS):
                Xc = Xbufs[(NS_ITERS + it) % 2]
                Xn = Xbufs[(NS_ITERS + it + 1) % 2]

                def post_h(t, g0, ps):
                    drain(HB[t][:, g0 * m:(g0 + 4) * m], ps[:], scale=-1.0)

                mm_groups(AAT, Xc, [0, 1], post_h, GRPS_A + GRPS_B)

                def inj_x(t, i, gi, ps, Xc=Xc):
                    nc.tensor.matmul(ps[:, gi * m:(gi + 1) * m], id2[:],
                                     Xc[t][:, i * m:(i + 1) * m],
                                     start=(gi % 2 == 0), stop=False)

                def post_x(t, g0, ps, Xn=Xn):
                    drain(Xn[t][:, g0 * m:(g0 + 4) * m], ps[:])

                mm_groups(Xc, HB, [0, 1], post_x, GRPS_A + GRPS_B, inject=inj_x)
            # M = MH (by construction of X0dst / ping-pong)

            # ---------- z0 with one refinement of h0 ----------
            # u = M g0; r = g0 - AAt u; h0 = u + M r; z0 = x - A^T h0
            NG = 2
            GW = I // NG
            GI = [range(g * GW, (g + 1) * GW) for g in range(NG)]
            SLI = [slice(g * GW * MT, (g + 1) * GW * MT) for g in range(NG)]
            GC = GW * MT
            psu = [None] * NG
            for g in range(NG):
                psu[g] = pv()
                mv_sym(psu[g], MH, g16, items=GI[g], base=GW * g)
                nc.vector.tensor_copy(h0v[:, SLI[g]], psu[g][:, 0:GC])
                nc.scalar.copy(u16[:, SLI[g]], psu[g][:, 0:GC])
            psv = [None] * NG
            for g in range(NG):
                psv[g] = pv()
                mv_sym(psv[g], AAT, u16, items=GI[g], base=GW * g)
                nc.vector.tensor_tensor(rtmp[:, SLI[g]], g0v[:, SLI[g]],
                                        psv[g][:, 0:GC], AluOpType.subtract)
                nc.scalar.copy(r16[:, SLI[g]], rtmp[:, SLI[g]])
            psd2 = [None] * NG
            for g in range(NG):
                psd2[g] = pv()
                mv_sym(psd2[g], MH, r16, items=GI[g], base=GW * g)
                nc.vector.tensor_tensor(h0v[:, SLI[g]], h0v[:, SLI[g]],
                                        psd2[g][:, 0:GC], AluOpType.add)
                nc.scalar.copy(h016[:, SLI[g]], h0v[:, SLI[g]])
            psz = pv()
            mv_up(psz, h016, range(SKT))
            nc.vector.tensor_tensor(z0m[:], xzv[:, 0:SI], psz[:, 0:SI], AluOpType.subtract)
            nc.vector.tensor_copy(zm[:], z0m[:])

            # ---------- active-set rounds ----------
            cur, oth = wa, wb
            for r, coeffs in enumerate(cheb_r):
                nc.vector.tensor_scalar(sig[:], zm[:], 0.0, None, AluOpType.is_lt)
                nc.vector.scalar_tensor_tensor(msig[:], sig[:], -1.0, m01v[:],
                                               AluOpType.mult, AluOpType.mult)
                nc.vector.tensor_tensor(tmpn[:], msig[:], z0m[:], AluOpType.mult)
                nc.scalar.activation(su16[:], tmpn[:], ACT_COPY, scale=-1.0)
                # S = AAt - A_sig A^T accumulated fully in PSUM:
                #   identity-weight matmul adds AAt, masked blocks add -A_sig A^T
                psd = None
                for g0 in (0, 4, 8, 12):
                    pss = [ps_p.tile([128, 1024], F32, name="psb", tag="psb")
                           for _ in range(MT)]
                    # identity-weight matmuls first: they add the AAt term and
                    # depend only on AAT, filling PE while sigma is computed
                    for gi in range(4):
                        i = g0 + gi
                        for t in range(MT):
                            nc.tensor.matmul(
                                pss[t][:, gi * m:(gi + 1) * m],
                                id128[:],
                                AAT[t][:, i * m:(i + 1) * m],
                                start=(gi % 2 == 0), stop=False)
                    if g0 == 0:
                        # rhs = A(sigma z0): streams on PE while DVE/Act build
                        # the first masked-scale tiles
                        psd = pv()
                        mv_dn(psd, su16, list(range(SKT)))
                        nc.vector.tensor_copy(rhsc[:], psd[:, 0:I * MT])
                    for gi in range(4):
                        i = g0 + gi
                        for kt in range(SKT):
                            mk16 = msk_p.tile([128, m], F16, name="mk", tag="mk")
                            if (gi * SKT + kt) % 5 < 3:
                                nc.vector.tensor_scalar(
                                    mk16[:], AT[kt][:, i * m:(i + 1) * m],
                                    msig[:, kt * I + i:kt * I + i + 1],
                                    None, AluOpType.mult)
                            else:
                                nc.scalar.activation(
                                    mk16[:], AT[kt][:, i * m:(i + 1) * m], ACT_COPY,
                                    scale=msig[:, kt * I + i:kt * I + i + 1])
                            for t in range(MT):
                                nc.tensor.matmul(
                                    pss[t][:, gi * m:(gi + 1) * m],
                                    mk16[:, t * 128:t * 128 + 128],
                                    AT[kt][:, i * m:(i + 1) * m],
                                    start=False,
                                    stop=(kt == SKT - 1 and gi % 2 == 1))
                    for t in range(MT):
                        sl = slice(g0 * m, (g0 + 4) * m)
                        drain(SH[t][:, sl], pss[t][:])
                if r == 0:
                    # gate l1u/xrow loads behind round-0 S so they never
                    # compete with the front-loaded at16/l1m traffic
                    for j in range(4):
                        nc.scalar.copy(LU[j][0:1, 0:1], SH[0][0:1, 0:1])
                    for g in range(2):
                        for rb in range(MT):
                            nc.sync.dma_start(
                                out=LU[g * MT + rb][:],
                                in_=l1u_d[rb][:, g * 8 * NU:(g + 1) * 8 * NU])
                # Chebyshev, two interleaved item groups; (cur, oth) ping-pong
                for k, (beta, gamma) in enumerate(coeffs):
                    for grp in range(2):
                        gb = grp * 8
                        sl = slice(gb * MT, (gb + 8) * MT)
                        nc.scalar.copy(w16[:, sl], cur[:, sl])
                        psk = pv()
                        mv_sym(psk, SH, w16, items=range(gb, gb + 8), base=gb)
                        nc.vector.tensor_tensor(wtmp[:, sl], cur[:, sl], oth[:, sl],
                                                AluOpType.subtract)
                        nc.vector.scalar_tensor_tensor(wtmp[:, sl], wtmp[:, sl], beta,
                                                       cur[:, sl], AluOpType.mult,
                                                       AluOpType.add)
                        nc.vector.tensor_tensor(rtmp[:, sl], rhsc[:, sl], psk[:, 0:16],
                                                AluOpType.subtract)
                        nc.vector.scalar_tensor_tensor(oth[:, sl], rtmp[:, sl], gamma,
                                                       wtmp[:, sl], AluOpType.mult,
                                                       AluOpType.add)
                    cur, oth = oth, cur
                # z = z0 + A^T w (masked)
                nc.scalar.copy(w16[:], cur[:])
                psz2 = pv()
                mv_up(psz2, w16, range(SKT))
                nc.vector.tensor_tensor(zm[:], z0m[:], psz2[:, 0:SI], AluOpType.add)

            # ---------- final ----------
            nc.vector.tensor_scalar(sig[:], zm[:], 0.0, None, AluOpType.is_lt)
            nc.vector.scalar_tensor_tensor(msig[:], sig[:], -1.0, m01v[:],
                                           AluOpType.mult, AluOpType.mult)
            nc.vector.tensor_tensor(tmpn[:], msig[:], zm[:], AluOpType.mult)
            nc.scalar.copy(nsz16[:], tmpn[:])
            # w16 still holds fp16(cur) from the last round's psz2 copy
            # g = AAt w - A(sigma z)  (one fused accumulation group per col)
            psg2 = pv()
            for i in range(I):
                for cb in range(MT):
                    col = i * MT + cb
                    for rb in range(MT):
                        nc.tensor.matmul(
                            psg2[:, col:col + 1],
                            AAT[rb][:, i * m + cb * 128: i * m + cb * 128 + 128],
                            w16[:, i * MT + rb: i * MT + rb + 1],
                            start=(rb == 0), stop=False)
                    for kt in range(SKT):
                        nc.tensor.matmul(
                            psg2[:, col:col + 1],
                            AT[kt][:, i * m + cb * 128: i * m + cb * 128 + 128],
                            nsz16[:, kt * I + i: kt * I + i + 1],
                            start=False, stop=(kt == SKT - 1))
            nc.scalar.copy(g16[:], psg2[:, 0:I * MT])
            psh2 = pv()
            mv_sym(psh2, MH, g16)
            nc.vector.tensor_tensor(wfv[:], cur[:], psh2[:, 0:I * MT], AluOpType.subtract)
            nc.vector.tensor_tensor(wfv[:], wfv[:], h0v[:], AluOpType.subtract)
            nc.scalar.copy(wf16[:], wfv[:])
            # out = x + A^T wf - sigma z  (full width)
            pso = pv()
            mv_up(pso, wf16, range(SKT))
            for g in range(2):
                for gi in range(8):
                    i = g * 8 + gi
                    for kj in range(UKT):
                        col = (SKT + kj) * I + i
                        for rb in range(MT):
                            nc.tensor.matmul(
                                pso[:, col:col + 1],
                                LU[g * MT + rb][:, gi * NU + kj * 128: gi * NU + kj * 128 + 128],
                                wf16[:, i * MT + rb: i * MT + rb + 1],
                                start=(rb == 0), stop=(rb == MT - 1))
            nc.vector.tensor_tensor(outv[:], xzv[:], pso[:, 0:KT * I], AluOpType.add)
            nc.vector.tensor_tensor(outv[:, 0:SI], outv[:, 0:SI], tmpn[:], AluOpType.add)
            nc.sync.dma_start(out=out_d[:], in_=outv[:])

        for _rep in range(reps):
            emit_body()

    nc.compile()
    return nc


def _prep_core(Ap, xp, bp, m01p, NM):
    A16 = Ap.astype(np.float16)  # [I, m, n]
    NU = n - NM
    SKT = NM // 128
    at = np.ascontiguousarray(A16.transpose(2, 0, 1)).reshape(KT, 128, IM)
    l1 = np.ascontiguousarray(A16.transpose(1, 0, 2))  # [m, I, n]
    l1m = np.ascontiguousarray(l1[:, :, :NM]).reshape(MT, 128, I * NM)
    l1u = np.ascontiguousarray(l1[:, :, NM:]).reshape(MT, 128, I * NU)
    xz = np.ascontiguousarray(
        xp.T.reshape(KT, 128, I).transpose(1, 0, 2)).reshape(128, KT * I)
    bc = np.ascontiguousarray(
        bp.reshape(I, MT, 128).transpose(2, 0, 1)).reshape(128, I * MT)
    m01 = np.ascontiguousarray(
        np.broadcast_to(m01p[:NM].reshape(SKT, 128, 1), (SKT, 128, I))
        .transpose(1, 0, 2)).reshape(128, SKT * I).astype(np.float32)
    return dict(at16=at, l1m=l1m, l1u=l1u,
                xz=np.ascontiguousarray(xz, dtype=np.float32),
                bc=np.ascontiguousarray(bc, dtype=np.float32), m01=m01)


_SHIMMED = False


def _fix_cc_flags():
    """Route static DMAs through SP so multi-wait DMAs are legal walrus
    codegen (the embedded-wait form only fits one sync wait)."""
    global _SHIMMED
    try:
        from concourse.compiler_utils import get_compiler_flags, set_compiler_flags
        flags = get_compiler_flags()
        nf = [f.replace("--assign-static-dmas-to-sp=false",
                        "--assign-static-dmas-to-sp=true") for f in flags]
        if nf != flags:
            set_compiler_flags(nf)
    except Exception:
        pass
    if not _SHIMMED:
        import concourse.bass_utils as BU
        orig = BU.run_command

        def patched(cmd, *a, **k):
            if isinstance(cmd, (list, tuple)):
                cmd = [str(c).replace("--assign-static-dmas-to-sp=false",
                                      "--assign-static-dmas-to-sp=true") for c in cmd]
            return orig(cmd, *a, **k)

        BU.run_command = patched
        _SHIMMED = True


def kernel(x, b, A, nonnegative_mask):
    from concourse.bass_utils import run_bass_kernel_spmd
    _fix_cc_flags()
    x = np.asarray(x, dtype=np.float32)
    b = np.asarray(b, dtype=np.float32)
    A = np.asarray(A, dtype=np.float32)
    mk = np.asarray(nonnegative_mask).astype(bool)

    perm = np.argsort(~mk, kind="stable")
    inv = np.argsort(perm, kind="stable")
    n_mk = int(mk.sum())
    NM = ((n_mk + 127) // 128) * 128
    Ap = A[:, :, perm]
    xp = x[:, perm]
    m01p = np.zeros(n, np.float32)
    m01p[:n_mk] = 1.0

    if n_mk not in _CACHE:
        _CACHE[n_mk] = _build(n_mk)
    nc = _CACHE[n_mk]

    in_maps = []
    for c in range(NCORES):
        s = slice(c * I, (c + 1) * I)
        in_maps.append(_prep_core(Ap[s], xp[s], b[s], m01p, NM))
    res = run_bass_kernel_spmd(nc, in_maps, core_ids=list(range(NCORES)))
    outs = []
    for r in res.results:
        o = r["out"].reshape(128, KT, I).transpose(2, 1, 0).reshape(I, n)
        outs.append(o)
    out_p = np.concatenate(outs, axis=0)
    return np.ascontiguousarray(out_p[:, inv]).astype(np.float32)
